# revision 21
# baseline (speedup 1.0000x reference)
"""DenseMaskPredictor Trainium2 kernel (bf16-output pipeline).

out[n] = paste(sigmoid(mask_output[n, cls[n]]), bbox[n]) onto a 768x768 canvas,
zero outside the box (bilinear, zero-padded sampling).

Math: the bilinear paste is separable:
    out_n[y, x] = sum_ij Wy[y,i] * probs_n[i,j] * Wx[x,j]
with W*[s, k] = relu(1 - a*|s - c_k|), c_k = (s0 - 0.5) + (k+0.5)*(s1-s0)/28,
a = 28/(s1-s0). Weights vanish outside the box, reproducing the reference's
zero-padded bilinear exactly; invalid classes get c = +1e9 -> all-zero canvas.

Device plan (per core, 16 instances as 4 groups of 4; instance b of a group
lives at partition block 32*b of every tile):
  - host precomputes (tiny [128, k] tensors from cls/bbox): per-group gather
    row offsets, and per-(group, axis) weight scalars -c_k / -a per partition.
  - one SWDGE indirect DMA per group gathers the class mask rows straight
    from DRAM into [128, 28] (partition 32b+i holds mask row i of instance b).
  - weights WyT/WxT [28(+4 pad), 768]: ScalarE Abs(iota - c) with per-partition
    bias, then ScalarE Relu(d * (-a) + 1) with per-partition scale -> bf16.
  - sigmoid on ScalarE -> bf16 probs.
  - V[j, y] = sum_i probs[i,j] WyT[i,y]: bf16 matmuls at tile position
    (32b, 32b); one merged [128, 768] copy evacuates to bf16 v_sb.
  - out[y, x] = sum_j V[j, ytile] WxT[j, x]: bf16 matmuls, 2 instances per
    PSUM tile [128, 1536] (chunks aligned to PSUM banks), evacuated by one
    [128, 1536] fp32->bf16 copy rotated across ScalarE/VectorE/GpSimd.
  - one 768KB HWDGE DMA per (group, y-tile) writes 4 instances' rows to DRAM.
  - 8 warmup matmuls at t=0 lift the PE HAM clock gate (1.2 -> 2.4 GHz)
    before the first real matmuls.

Output is written bf16 (PSUM accumulates fp32; only the final store rounds),
upcast to fp32 on host. Data-parallel over N=128 instances across 8 cores.
"""

import os
import sys

import numpy as np

for _p in ("/opt/trn_rl_repo",):
    if _p not in sys.path and os.path.isdir(_p):
        sys.path.insert(0, _p)

N_FULL = 128
N_CORES = 8
N_LOC = N_FULL // N_CORES  # 16 instances per core
C = 80
M = 28
H = W = 768
NUM_VALID = 80
GROUPS = N_LOC // 4  # groups of 4 instances
TILES = H // 128  # 6 y-tiles of 128 rows


def _emit(tc, nc, masks, offs, wvals, out):
    import concourse.bass as bass
    from concourse import mybir

    f32 = mybir.dt.float32
    bf16 = mybir.dt.bfloat16
    i32 = mybir.dt.int32
    AF = mybir.ActivationFunctionType
    OP = mybir.AluOpType
    ctx = tc._emit_ctx  # ExitStack supplied by caller

    const = ctx.enter_context(tc.tile_pool(name="const", bufs=1))
    ppool = ctx.enter_context(tc.tile_pool(name="ppool", bufs=2))
    gpool = ctx.enter_context(tc.tile_pool(name="gpool", bufs=2))
    wpool = ctx.enter_context(tc.tile_pool(name="wpool", bufs=4))
    vpool = ctx.enter_context(tc.tile_pool(name="vpool", bufs=2))
    stage = ctx.enter_context(tc.tile_pool(name="stage", bufs=6))
    ps_v = ctx.enter_context(tc.tile_pool(name="ps_v", bufs=1, space="PSUM"))
    ps_o = ctx.enter_context(tc.tile_pool(name="ps_o", bufs=3, space="PSUM"))

    # ---------------- inputs (host-precomputed scalars) ----------------
    offs_sb = const.tile([128, GROUPS], i32)
    nc.sync.dma_start(offs_sb[:, :], offs[:, :])
    wvals_sb = const.tile([128, 4 * GROUPS], f32)
    nc.sync.dma_start(wvals_sb[:, :], wvals[:, :])

    # preload all ACT function tables off the critical path: the first real
    # Abs/Relu/Sigmoid otherwise each eat a ~1.5us ACT_TABLE_LOAD mid-ramp
    tiny = const.tile([128, 1], f32)
    nc.vector.memset(tiny[:, :], 0.0)
    warm_act = const.tile([128, 1], f32)
    nc.scalar.activation(warm_act[:, :], tiny[:, :], AF.Abs)
    nc.scalar.activation(warm_act[:, :], tiny[:, :], AF.Sigmoid)
    nc.scalar.activation(warm_act[:, :], tiny[:, :], AF.Relu)

    # ---------------- constants ----------------
    iota_i = const.tile([128, W], i32)
    nc.gpsimd.iota(iota_i[:, :], pattern=[[1, W]], channel_multiplier=0)
    iota_f = const.tile([128, W], f32)
    nc.vector.tensor_copy(iota_f[:, :], iota_i[:, :])

    # PE warmup: HAM un-throttles after ~3.4us of sustained activity; these
    # dummies run during the gather phase so real matmuls start at 2.4 GHz.
    warm_sb = const.tile([128, 512], bf16)
    nc.vector.memset(warm_sb[:, :], 0.0)
    warm_ps = ps_v.tile([128, W], f32, tag="v_ps", name="warm")
    for _ in range(8):
        nc.tensor.matmul(
            out=warm_ps[:, 0:512],
            lhsT=warm_sb[:, 0:128],
            rhs=warm_sb[:, :],
            start=True,
            stop=True,
        )

    # ---------------- class-mask gathers (one indirect DMA per group) ------
    # masks viewed as rows of 28 floats; offs[p, g] selects DRAM row
    # (n*C + clip(cls_n))*28 + min(p%32, 27) for instance n = 4g + p//32.
    masks_rows = masks.rearrange("n c h w -> (n c h) w")
    probs_pre = [
        const.tile([128, M], f32, name=f"probs_pre{g}") for g in range(GROUPS)
    ]

    def gather(g):
        nc.gpsimd.indirect_dma_start(
            out=probs_pre[g][:, :],
            out_offset=None,
            in_=masks_rows,
            in_offset=bass.IndirectOffsetOnAxis(ap=offs_sb[:, g : g + 1], axis=0),
        )

    for g in range(GROUPS):
        gather(g)

    V_CH = ((0, 512), (512, 256))  # N-chunks that stay inside one PSUM bank

    # PSUM evacuation is ScalarE/VectorE only (GpSimd cannot access PSUM).
    # Greedy time-balanced assignment: ScalarE reads PSUM faster (~0.87us
    # per [128,768] vs ~1.05 on DVE) but also owns the weight builds.
    eng_clock = [0.0, 0.0]  # scalar, vector

    def copy_psum(dst, src, cost_sc, cost_ve):
        if eng_clock[0] <= eng_clock[1]:
            eng_clock[0] += cost_sc
            nc.scalar.copy(dst, src)
        else:
            eng_clock[1] += cost_ve
            nc.vector.tensor_copy(dst, src)

    # ---------------- per-group pipeline ----------------
    for g in range(GROUPS):
        # interpolation weight tiles: w = relu(1 - a*|s - c|) per partition,
        # built on VectorE (fp32 SBUF tensor_scalar runs at 2 elems/cycle);
        # sigmoid on ScalarE so the V matmuls only wait on the gather.
        probs = ppool.tile([128, M], bf16, tag="probs")
        nc.scalar.activation(probs[:, :], probs_pre[g][:, :], AF.Sigmoid)
        w_tiles = []
        for qi in range(2):  # 0 = y axis, 1 = x axis
            ncol = wvals_sb[:, 4 * g + 2 * qi : 4 * g + 2 * qi + 1]  # -c
            acol = wvals_sb[:, 4 * g + 2 * qi + 1 : 4 * g + 2 * qi + 2]  # -a
            d_t = gpool.tile([128, W], f32, tag=f"d{qi}")
            nc.scalar.activation(d_t[:, :], iota_f[:, :], AF.Abs, bias=ncol)
            w_t = wpool.tile([128, W], bf16, tag=f"w{qi}")
            nc.scalar.activation(w_t[:, :], d_t[:, :], AF.Relu, bias=1.0, scale=acol)
            w_tiles.append(w_t)
        w_y, w_x = w_tiles
        eng_clock[0] += 4.2 + 0.3  # weight builds + sigmoid land on ScalarE

        # V[j, y] = sum_i probs[i, j] * WyT[i, y]
        v_ps = ps_v.tile([128, W], f32, tag="v_ps")
        for b in range(4):
            for (c0, cn) in V_CH:
                nc.tensor.matmul(
                    out=v_ps[32 * b : 32 * b + M, c0 : c0 + cn],
                    lhsT=probs[32 * b : 32 * b + M, :],
                    rhs=w_y[32 * b : 32 * b + M, c0 : c0 + cn],
                    start=True,
                    stop=True,
                    tile_position=(32 * b, 32 * b),
                )
        # split the V evacuation across both PSUM-capable engines
        v_sb = vpool.tile([128, W], bf16, tag="v_sb")
        nc.scalar.copy(v_sb[:, : W // 2], v_ps[:, : W // 2])
        nc.vector.tensor_copy(v_sb[:, W // 2 :], v_ps[:, W // 2 :])
        eng_clock[0] += 0.45
        eng_clock[1] += 0.55

        # out[y, x] = sum_j V[j, y] * WxT[j, x]; one PSUM tile per instance
        # (3 bufs) so next-tile matmuls never wait on this tile's evacuation
        for t in range(TILES):
            st = stage.tile([128, 4 * W], bf16, tag="st")
            for b in range(4):
                o_ps = ps_o.tile([128, W], f32, tag="o_ps")
                for (c0, cn) in V_CH:
                    nc.tensor.matmul(
                        out=o_ps[:, c0 : c0 + cn],
                        lhsT=v_sb[32 * b : 32 * b + M, t * 128 : (t + 1) * 128],
                        rhs=w_x[32 * b : 32 * b + M, c0 : c0 + cn],
                        start=True,
                        stop=True,
                        tile_position=(32 * b, 0),
                    )
                copy_psum(st[:, b * W : (b + 1) * W], o_ps[:, :], 0.87, 1.05)
            nc.sync.dma_start(
                out[4 * g : 4 * g + 4, t * 128 : (t + 1) * 128, :].rearrange(
                    "n y x -> y n x"
                ),
                st[:, :],
            )


def _build_program():
    import concourse.tile as tile
    from concourse import bacc, mybir
    from contextlib import ExitStack

    f32 = mybir.dt.float32
    bf16 = mybir.dt.bfloat16
    i32 = mybir.dt.int32

    nc = bacc.Bacc("TRN2", target_bir_lowering=False, debug=False)
    masks = nc.dram_tensor("masks", [N_LOC, C, M, M], f32, kind="ExternalInput").ap()
    offs = nc.dram_tensor("offs", [128, GROUPS], i32, kind="ExternalInput").ap()
    wvals = nc.dram_tensor("wvals", [128, 4 * GROUPS], f32, kind="ExternalInput").ap()
    out = nc.dram_tensor("out", [N_LOC, H, W], bf16, kind="ExternalOutput").ap()

    with tile.TileContext(nc) as tc:
        with ExitStack() as ctx:
            tc._emit_ctx = ctx
            _emit(tc, nc, masks, offs, wvals, out)
    nc.compile()
    return nc


_NC = None


def _get_program():
    global _NC
    if _NC is None:
        _NC = _build_program()
    return _NC


def _host_scalars(cls16, bbox16):
    """Per-core [128, k] tensors: gather row offsets + weight scalars."""
    p = np.arange(128)
    b = p // 32  # instance-in-group
    k = p % 32  # mask row / interp index per partition
    kcl = np.minimum(k, M - 1)

    cls = cls16.astype(np.int64)
    valid = (cls >= 0) & (cls < NUM_VALID)
    ccl = np.clip(cls, 0, C - 1)
    row_base = (np.arange(N_LOC) * C + ccl) * M  # [16]

    offs = np.empty((128, GROUPS), dtype=np.int32)
    wvals = np.empty((128, 4 * GROUPS), dtype=np.float32)
    pad = k >= M
    for g in range(GROUPS):
        n = 4 * g + b  # [128] instance ids
        offs[:, g] = row_base[n] + kcl
        for qi, (c0i, c1i) in enumerate(((1, 3), (0, 2))):  # y=(y0,y1), x=(x0,x1)
            s0 = bbox16[n, c0i]
            s1 = bbox16[n, c1i]
            ra = (s1 - s0) / M
            ck = (s0 - 0.5) + (k + 0.5) * ra
            ck = np.where(pad | ~valid[n], 1.0e9, ck)
            wvals[:, 4 * g + 2 * qi] = -ck
            wvals[:, 4 * g + 2 * qi + 1] = -M / (s1 - s0)
    return offs, wvals


def make_in_maps(mask_output, class_indices, bbox_tensor):
    mask_output = np.asarray(mask_output, dtype=np.float32)
    class_indices = np.asarray(class_indices)
    bbox_tensor = np.asarray(bbox_tensor, dtype=np.float32)
    in_maps = []
    for cidx in range(N_CORES):
        sl = slice(cidx * N_LOC, (cidx + 1) * N_LOC)
        offs, wvals = _host_scalars(class_indices[sl], bbox_tensor[sl])
        in_maps.append(
            {
                "masks": np.ascontiguousarray(mask_output[sl]),
                "offs": offs,
                "wvals": wvals,
            }
        )
    return in_maps


def kernel(mask_output, class_indices, bbox_tensor, scene_h=H, scene_w=W, **kwargs):
    assert int(scene_h) == H and int(scene_w) == W
    from concourse.bass_utils import run_bass_kernel_spmd

    nc = _get_program()
    in_maps = make_in_maps(mask_output, class_indices, bbox_tensor)
    res = run_bass_kernel_spmd(nc, in_maps, list(range(N_CORES)))
    out = np.concatenate([np.asarray(r["out"]) for r in res.results], axis=0)
    return out.astype(np.float32)


# revision 31
# speedup vs baseline: 1.1311x; 1.1311x over previous
"""DenseMaskPredictor Trainium2 kernel (bf16-output pipeline).

out[n] = paste(sigmoid(mask_output[n, cls[n]]), bbox[n]) onto a 768x768 canvas,
zero outside the box (bilinear, zero-padded sampling).

Math: the bilinear paste is separable:
    out_n[y, x] = sum_ij Wy[y,i] * probs_n[i,j] * Wx[x,j]
with W*[s, k] = relu(1 - a*|s - c_k|), c_k = (s0 - 0.5) + (k+0.5)*(s1-s0)/28,
a = 28/(s1-s0). Weights vanish outside the box, reproducing the reference's
zero-padded bilinear exactly; invalid classes get c = +1e9 -> all-zero canvas.

Device plan (per core, 16 instances as 4 groups of 4; instance b of a group
lives at partition block 32*b of every tile):
  - host precomputes (tiny [128, k] tensors from cls/bbox): per-group gather
    row offsets, and per-(group, axis) weight scalars -c_k / -a per partition.
  - one SWDGE indirect DMA per group gathers the class mask rows straight
    from DRAM into [128, 28] (partition 32b+i holds mask row i of instance b).
  - weights WyT/WxT [28(+4 pad), 768]: ScalarE Abs(iota - c) with per-partition
    bias, then ScalarE Relu(d * (-a) + 1) with per-partition scale -> bf16.
  - sigmoid on ScalarE -> bf16 probs.
  - V[j, y] = sum_i probs[i,j] WyT[i,y]: bf16 matmuls at tile position
    (32b, 32b); one merged [128, 768] copy evacuates to bf16 v_sb.
  - out[y, x] = sum_j V[j, ytile] WxT[j, x]: bf16 matmuls, 2 instances per
    PSUM tile [128, 1536] (chunks aligned to PSUM banks), evacuated by one
    [128, 1536] fp32->bf16 copy rotated across ScalarE/VectorE/GpSimd.
  - one 768KB HWDGE DMA per (group, y-tile) writes 4 instances' rows to DRAM.
  - 8 warmup matmuls at t=0 lift the PE HAM clock gate (1.2 -> 2.4 GHz)
    before the first real matmuls.

Output is written bf16 (PSUM accumulates fp32; only the final store rounds),
upcast to fp32 on host. Data-parallel over N=128 instances across 8 cores.
"""

import os
import sys

import numpy as np

for _p in ("/opt/trn_rl_repo",):
    if _p not in sys.path and os.path.isdir(_p):
        sys.path.insert(0, _p)

N_FULL = 128
N_CORES = 8
N_LOC = N_FULL // N_CORES  # 16 instances per core
C = 80
M = 28
H = W = 768
NUM_VALID = 80
GROUPS = N_LOC // 4  # groups of 4 instances
TILES = H // 128  # 6 y-tiles of 128 rows


def _emit(tc, nc, masks, offs, wvals, out):
    import concourse.bass as bass
    from concourse import mybir

    f32 = mybir.dt.float32
    bf16 = mybir.dt.bfloat16
    i32 = mybir.dt.int32
    AF = mybir.ActivationFunctionType
    OP = mybir.AluOpType
    ctx = tc._emit_ctx  # ExitStack supplied by caller

    const = ctx.enter_context(tc.tile_pool(name="const", bufs=1))
    ppool = ctx.enter_context(tc.tile_pool(name="ppool", bufs=2))
    gpool = ctx.enter_context(tc.tile_pool(name="gpool", bufs=2))
    wpool = ctx.enter_context(tc.tile_pool(name="wpool", bufs=4))
    vpool = ctx.enter_context(tc.tile_pool(name="vpool", bufs=2))
    stage = ctx.enter_context(tc.tile_pool(name="stage", bufs=6))
    ps_v = ctx.enter_context(tc.tile_pool(name="ps_v", bufs=1, space="PSUM"))
    ps_o = ctx.enter_context(tc.tile_pool(name="ps_o", bufs=3, space="PSUM"))

    # ---------------- inputs (host-precomputed scalars) ----------------
    offs_sb = const.tile([128, GROUPS], i32)
    nc.sync.dma_start(offs_sb[:, :], offs[:, :])
    wvals_sb = const.tile([128, 8 * GROUPS], f32)
    nc.sync.dma_start(wvals_sb[:, :], wvals[:, :])

    # preload the ACT function table off the critical path: the first real
    # sigmoid otherwise eats a ~1.5us ACT_TABLE_LOAD mid-ramp
    tiny = const.tile([128, 1], f32)
    nc.vector.memset(tiny[:, :], 0.0)
    warm_act = const.tile([128, 1], f32)
    nc.scalar.activation(warm_act[:, :], tiny[:, :], AF.Sigmoid)

    # ---------------- constants ----------------
    iota_i = const.tile([128, W], i32)
    nc.gpsimd.iota(iota_i[:, :], pattern=[[1, W]], channel_multiplier=0)
    iota_f = const.tile([128, W], f32)
    nc.vector.tensor_copy(iota_f[:, :], iota_i[:, :])

    # PE warmup: HAM un-throttles after ~3.4us of sustained activity; these
    # dummies run during the gather phase so real matmuls start at 2.4 GHz.
    warm_sb = const.tile([128, 512], bf16)
    nc.vector.memset(warm_sb[:, :], 0.0)
    warm_ps = ps_v.tile([128, W], f32, tag="v_ps", name="warm")
    for _ in range(8):
        nc.tensor.matmul(
            out=warm_ps[:, 0:512],
            lhsT=warm_sb[:, 0:128],
            rhs=warm_sb[:, :],
            start=True,
            stop=True,
        )

    # ---------------- class-mask gathers (one indirect DMA per group) ------
    # masks viewed as rows of 28 floats; offs[p, g] selects DRAM row
    # (n*C + clip(cls_n))*28 + min(p%32, 27) for instance n = 4g + p//32.
    masks_rows = masks.rearrange("n c h w -> (n c h) w")
    probs_pre = [
        const.tile([128, M], f32, name=f"probs_pre{g}") for g in range(GROUPS)
    ]

    def gather(g):
        nc.gpsimd.indirect_dma_start(
            out=probs_pre[g][:, :],
            out_offset=None,
            in_=masks_rows,
            in_offset=bass.IndirectOffsetOnAxis(ap=offs_sb[:, g : g + 1], axis=0),
        )

    # group 0's gather leads the Q7 queue; 1-3 are emitted after group 0's
    # weight build (their data isn't needed until much later)
    gather(0)

    V_CH = ((0, 512), (512, 256))  # N-chunks that stay inside one PSUM bank

    # PSUM evacuation is ScalarE/VectorE only (GpSimd cannot access PSUM).
    # Greedy time-balanced assignment: ScalarE reads PSUM faster (~0.87us
    # per [128,768] vs ~1.05 on DVE) but also owns the weight builds.
    eng_clock = [0.0, 0.0]  # scalar, vector

    def copy_psum(dst, src, cost_sc, cost_ve):
        if eng_clock[0] <= eng_clock[1]:
            eng_clock[0] += cost_sc
            nc.scalar.copy(dst, src)
        else:
            eng_clock[1] += cost_ve
            nc.vector.tensor_copy(dst, src)

    # ---------------- per-group pipeline ----------------
    for g in range(GROUPS):
        # interpolation weight tiles: w = relu(1 - a*|s - c|) per partition,
        # built on VectorE (fp32 SBUF tensor_scalar runs at 2 elems/cycle);
        # sigmoid on ScalarE so the V matmuls only wait on the gather.
        probs = ppool.tile([128, M], bf16, tag="probs")
        nc.scalar.activation(probs[:, :], probs_pre[g][:, :], AF.Sigmoid)
        # w = relu(min(u, v)) with u = 1 - a*(s - c), v = 1 + a*(s - c):
        # u/v/relu on VectorE (2x-mode fp32 tensor_scalar), min on GpSimd
        # (tensor_tensor, SBUF-only) -- ScalarE keeps only sigmoid + copies.
        w_tiles = []
        for qi in range(2):  # 0 = y axis, 1 = x axis
            cb = 8 * g + 4 * qi
            u_t = gpool.tile([128, W], f32, tag=f"u{qi}")
            nc.vector.tensor_scalar(
                u_t[:, :], iota_f[:, :], wvals_sb[:, cb : cb + 1],
                wvals_sb[:, cb + 1 : cb + 2], op0=OP.mult, op1=OP.add,
            )
            v_t = gpool.tile([128, W], f32, tag=f"v{qi}")
            nc.vector.tensor_scalar(
                v_t[:, :], iota_f[:, :], wvals_sb[:, cb + 2 : cb + 3],
                wvals_sb[:, cb + 3 : cb + 4], op0=OP.mult, op1=OP.add,
            )
            m_t = gpool.tile([128, W], f32, tag=f"m{qi}")
            nc.vector.tensor_tensor(m_t[:, :], u_t[:, :], v_t[:, :], op=OP.min)
            w_t = wpool.tile([128, W], bf16, tag=f"w{qi}")
            nc.vector.tensor_scalar(w_t[:, :], m_t[:, :], 0.0, None, op0=OP.max)
            w_tiles.append(w_t)
        w_y, w_x = w_tiles
        eng_clock[0] += 0.3  # sigmoid on ScalarE
        eng_clock[1] += 4.5  # u/v/min/relu on VectorE
        if g == 0:
            for gg in range(1, GROUPS):
                gather(gg)

        # V[j, y] = sum_i probs[i, j] * WyT[i, y]
        v_ps = ps_v.tile([128, W], f32, tag="v_ps")
        for b in range(4):
            for (c0, cn) in V_CH:
                nc.tensor.matmul(
                    out=v_ps[32 * b : 32 * b + M, c0 : c0 + cn],
                    lhsT=probs[32 * b : 32 * b + M, :],
                    rhs=w_y[32 * b : 32 * b + M, c0 : c0 + cn],
                    start=True,
                    stop=True,
                    tile_position=(32 * b, 32 * b),
                )
        # split the V evacuation across both PSUM-capable engines
        v_sb = vpool.tile([128, W], bf16, tag="v_sb")
        nc.scalar.copy(v_sb[:, : W // 2], v_ps[:, : W // 2])
        nc.vector.tensor_copy(v_sb[:, W // 2 :], v_ps[:, W // 2 :])
        eng_clock[0] += 0.33
        eng_clock[1] += 0.58

        # out[y, x] = sum_j V[j, y] * WxT[j, x]; one PSUM tile per instance
        # (3 bufs) so next-tile matmuls never wait on this tile's evacuation
        for t in range(TILES):
            st = stage.tile([128, 4 * W], bf16, tag="st")
            for b in range(4):
                o_ps = ps_o.tile([128, W], f32, tag="o_ps")
                for (c0, cn) in V_CH:
                    nc.tensor.matmul(
                        out=o_ps[:, c0 : c0 + cn],
                        lhsT=v_sb[32 * b : 32 * b + M, t * 128 : (t + 1) * 128],
                        rhs=w_x[32 * b : 32 * b + M, c0 : c0 + cn],
                        start=True,
                        stop=True,
                        tile_position=(32 * b, 0),
                    )
                copy_psum(st[:, b * W : (b + 1) * W], o_ps[:, :], 0.57, 1.08)
            nc.sync.dma_start(
                out[4 * g : 4 * g + 4, t * 128 : (t + 1) * 128, :].rearrange(
                    "n y x -> y n x"
                ),
                st[:, :],
            )


def _build_program():
    import concourse.tile as tile
    from concourse import bacc, mybir
    from contextlib import ExitStack

    f32 = mybir.dt.float32
    bf16 = mybir.dt.bfloat16
    i32 = mybir.dt.int32

    nc = bacc.Bacc("TRN2", target_bir_lowering=False, debug=False)
    masks = nc.dram_tensor("masks", [N_LOC, C, M, M], f32, kind="ExternalInput").ap()
    offs = nc.dram_tensor("offs", [128, GROUPS], i32, kind="ExternalInput").ap()
    wvals = nc.dram_tensor("wvals", [128, 8 * GROUPS], f32, kind="ExternalInput").ap()
    out = nc.dram_tensor("out", [N_LOC, H, W], bf16, kind="ExternalOutput").ap()

    with tile.TileContext(nc) as tc:
        with ExitStack() as ctx:
            tc._emit_ctx = ctx
            _emit(tc, nc, masks, offs, wvals, out)
    nc.compile()
    return nc


_NC = None


def _get_program():
    global _NC
    if _NC is None:
        _NC = _build_program()
    return _NC


def _host_scalars(cls16, bbox16):
    """Per-core [128, k] tensors: gather row offsets + weight scalars."""
    p = np.arange(128)
    b = p // 32  # instance-in-group
    k = p % 32  # mask row / interp index per partition
    kcl = np.minimum(k, M - 1)

    cls = cls16.astype(np.int64)
    valid = (cls >= 0) & (cls < NUM_VALID)
    ccl = np.clip(cls, 0, C - 1)
    row_base = (np.arange(N_LOC) * C + ccl) * M  # [16]

    offs = np.empty((128, GROUPS), dtype=np.int32)
    wvals = np.empty((128, 8 * GROUPS), dtype=np.float32)
    pad = k >= M
    for g in range(GROUPS):
        n = 4 * g + b  # [128] instance ids
        offs[:, g] = row_base[n] + kcl
        for qi, (c0i, c1i) in enumerate(((1, 3), (0, 2))):  # y=(y0,y1), x=(x0,x1)
            s0 = bbox16[n, c0i]
            s1 = bbox16[n, c1i]
            ra = (s1 - s0) / M
            a = M / (s1 - s0)
            ck = (s0 - 0.5) + (k + 0.5) * ra
            ck = np.where(pad | ~valid[n], 1.0e9, ck)
            cb = 8 * g + 4 * qi
            # w = relu(min(u, v)); u = -a*s + (1 + a*c), v = a*s + (1 - a*c)
            wvals[:, cb + 0] = -a
            wvals[:, cb + 1] = 1.0 + a * ck
            wvals[:, cb + 2] = a
            wvals[:, cb + 3] = 1.0 - a * ck
    return offs, wvals


def make_in_maps(mask_output, class_indices, bbox_tensor):
    mask_output = np.asarray(mask_output, dtype=np.float32)
    class_indices = np.asarray(class_indices)
    bbox_tensor = np.asarray(bbox_tensor, dtype=np.float32)
    in_maps = []
    for cidx in range(N_CORES):
        sl = slice(cidx * N_LOC, (cidx + 1) * N_LOC)
        offs, wvals = _host_scalars(class_indices[sl], bbox_tensor[sl])
        in_maps.append(
            {
                "masks": np.ascontiguousarray(mask_output[sl]),
                "offs": offs,
                "wvals": wvals,
            }
        )
    return in_maps


def kernel(mask_output, class_indices, bbox_tensor, scene_h=H, scene_w=W, **kwargs):
    assert int(scene_h) == H and int(scene_w) == W
    from concourse.bass_utils import run_bass_kernel_spmd

    nc = _get_program()
    in_maps = make_in_maps(mask_output, class_indices, bbox_tensor)
    res = run_bass_kernel_spmd(nc, in_maps, list(range(N_CORES)))
    out = np.concatenate([np.asarray(r["out"]) for r in res.results], axis=0)
    return out.astype(np.float32)


# revision 39
# speedup vs baseline: 1.1605x; 1.0260x over previous
"""DenseMaskPredictor Trainium2 kernel (bf16-output pipeline).

out[n] = paste(sigmoid(mask_output[n, cls[n]]), bbox[n]) onto a 768x768 canvas,
zero outside the box (bilinear, zero-padded sampling).

Math: the bilinear paste is separable:
    out_n[y, x] = sum_ij Wy[y,i] * probs_n[i,j] * Wx[x,j]
with W*[s, k] = relu(1 - a*|s - c_k|), c_k = (s0 - 0.5) + (k+0.5)*(s1-s0)/28,
a = 28/(s1-s0). Weights vanish outside the box, reproducing the reference's
zero-padded bilinear exactly; invalid classes get c = +1e9 -> all-zero canvas.

Device plan (per core, 16 instances as 4 groups of 4; instance b of a group
lives at partition block 32*b of every tile):
  - host precomputes (tiny [128, k] tensors from cls/bbox): per-group gather
    row offsets, and per-(group, axis) weight scalars -c_k / -a per partition.
  - one SWDGE indirect DMA per group gathers the class mask rows straight
    from DRAM into [128, 28] (partition 32b+i holds mask row i of instance b).
  - weights WyT/WxT [28(+4 pad), 768]: ScalarE Abs(iota - c) with per-partition
    bias, then ScalarE Relu(d * (-a) + 1) with per-partition scale -> bf16.
  - sigmoid on ScalarE -> bf16 probs.
  - V[j, y] = sum_i probs[i,j] WyT[i,y]: bf16 matmuls at tile position
    (32b, 32b); one merged [128, 768] copy evacuates to bf16 v_sb.
  - out[y, x] = sum_j V[j, ytile] WxT[j, x]: bf16 matmuls, 2 instances per
    PSUM tile [128, 1536] (chunks aligned to PSUM banks), evacuated by one
    [128, 1536] fp32->bf16 copy rotated across ScalarE/VectorE/GpSimd.
  - one 768KB HWDGE DMA per (group, y-tile) writes 4 instances' rows to DRAM.
  - 8 warmup matmuls at t=0 lift the PE HAM clock gate (1.2 -> 2.4 GHz)
    before the first real matmuls.

Output is written bf16 (PSUM accumulates fp32; only the final store rounds),
upcast to fp32 on host. Data-parallel over N=128 instances across 8 cores.
"""

import os
import sys

import numpy as np

for _p in ("/opt/trn_rl_repo",):
    if _p not in sys.path and os.path.isdir(_p):
        sys.path.insert(0, _p)

N_FULL = 128
N_CORES = 8
N_LOC = N_FULL // N_CORES  # 16 instances per core
C = 80
M = 28
H = W = 768
NUM_VALID = 80
GROUPS = N_LOC // 4  # groups of 4 instances
TILES = H // 128  # 6 y-tiles of 128 rows


def _emit(tc, nc, masks, offs, wvals, iota, out):
    import concourse.bass as bass
    from concourse import mybir

    f32 = mybir.dt.float32
    bf16 = mybir.dt.bfloat16
    i32 = mybir.dt.int32
    AF = mybir.ActivationFunctionType
    OP = mybir.AluOpType
    ctx = tc._emit_ctx  # ExitStack supplied by caller

    const = ctx.enter_context(tc.tile_pool(name="const", bufs=1))
    ppool = ctx.enter_context(tc.tile_pool(name="ppool", bufs=2))
    gpool = ctx.enter_context(tc.tile_pool(name="gpool", bufs=2))
    wpool = ctx.enter_context(tc.tile_pool(name="wpool", bufs=4))
    vpool = ctx.enter_context(tc.tile_pool(name="vpool", bufs=2))
    stage = ctx.enter_context(tc.tile_pool(name="stage", bufs=6))
    ps_v = ctx.enter_context(tc.tile_pool(name="ps_v", bufs=1, space="PSUM"))
    ps_o = ctx.enter_context(tc.tile_pool(name="ps_o", bufs=3, space="PSUM"))

    # ---------------- inputs (host-precomputed scalars) ----------------
    offs_sb = const.tile([128, GROUPS], i32)
    nc.sync.dma_start(offs_sb[:, :], offs[:, :])
    wvals_sb = const.tile([128, 8 * GROUPS], f32)
    nc.sync.dma_start(wvals_sb[:, :], wvals[:, :])
    # iota comes from the host too: keeps the GpSimd queue free for the mask
    # gathers and saves an on-chip iota+cast during the ramp
    iota_f = const.tile([128, W], f32)
    nc.sync.dma_start(iota_f[:, :], iota[:, :])

    # preload the ACT function table off the critical path: the first real
    # sigmoid otherwise eats a ~1.5us ACT_TABLE_LOAD mid-ramp
    tiny = const.tile([128, 1], f32)
    nc.vector.memset(tiny[:, :], 0.0)
    warm_act = const.tile([128, 1], f32)
    nc.scalar.activation(warm_act[:, :], tiny[:, :], AF.Sigmoid)

    # PE warmup: HAM un-throttles after ~3.4us of sustained activity; these
    # dummies run during the gather phase so real matmuls start at 2.4 GHz.
    warm_sb = const.tile([128, 512], bf16)
    nc.vector.memset(warm_sb[:, :], 0.0)
    warm_ps = ps_v.tile([128, W], f32, tag="v_ps", name="warm")
    for _ in range(8):
        nc.tensor.matmul(
            out=warm_ps[:, 0:512],
            lhsT=warm_sb[:, 0:128],
            rhs=warm_sb[:, :],
            start=True,
            stop=True,
        )

    # ---------------- class-mask gathers (one indirect DMA per group) ------
    # masks viewed as rows of 28 floats; offs[p, g] selects DRAM row
    # (n*C + clip(cls_n))*28 + min(p%32, 27) for instance n = 4g + p//32.
    masks_rows = masks.rearrange("n c h w -> (n c h) w")
    probs_pre = [
        const.tile([128, M], f32, name=f"probs_pre{g}") for g in range(GROUPS)
    ]

    def gather(g):
        nc.gpsimd.indirect_dma_start(
            out=probs_pre[g][:, :],
            out_offset=None,
            in_=masks_rows,
            in_offset=bass.IndirectOffsetOnAxis(ap=offs_sb[:, g : g + 1], axis=0),
        )

    # group 0's gather leads the Q7 queue; 1-3 are emitted after group 0's
    # weight build (their data isn't needed until much later)
    gather(0)

    V_CH = ((0, 512), (512, 256))  # N-chunks that stay inside one PSUM bank

    # PSUM evacuation is ScalarE/VectorE only (GpSimd cannot access PSUM).
    # Greedy time-balanced assignment: ScalarE reads PSUM faster (~0.87us
    # per [128,768] vs ~1.05 on DVE) but also owns the weight builds.
    eng_clock = [0.0, 0.0]  # scalar, vector

    def copy_psum(dst, src, cost_sc, cost_ve):
        if eng_clock[0] <= eng_clock[1]:
            eng_clock[0] += cost_sc
            nc.scalar.copy(dst, src)
        else:
            eng_clock[1] += cost_ve
            nc.vector.tensor_copy(dst, src)

    # ---------------- per-group pipeline ----------------
    for g in range(GROUPS):
        # interpolation weight tiles: w = relu(1 - a*|s - c|) per partition,
        # built on VectorE (fp32 SBUF tensor_scalar runs at 2 elems/cycle);
        # sigmoid on ScalarE so the V matmuls only wait on the gather.
        probs = ppool.tile([128, M], bf16, tag="probs")
        nc.scalar.activation(probs[:, :], probs_pre[g][:, :], AF.Sigmoid)
        # w = relu(min(u, v)) with u = 1 - a*(s - c), v = 1 + a*(s - c):
        # u on VectorE (2x-mode fp32 tensor_scalar), v on ScalarE (Copy
        # activation with per-partition scale/bias), min + relu on VectorE.
        w_tiles = []
        for qi in range(2):  # 0 = y axis, 1 = x axis
            cb = 8 * g + 4 * qi
            u_t = gpool.tile([128, W], f32, tag=f"u{qi}")
            nc.vector.tensor_scalar(
                u_t[:, :], iota_f[:, :], wvals_sb[:, cb : cb + 1],
                wvals_sb[:, cb + 1 : cb + 2], op0=OP.mult, op1=OP.add,
            )
            v_t = gpool.tile([128, W], f32, tag=f"v{qi}")
            nc.scalar.activation(
                v_t[:, :], iota_f[:, :], AF.Identity,
                bias=wvals_sb[:, cb + 3 : cb + 4],
                scale=wvals_sb[:, cb + 2 : cb + 3],
            )
            m_t = gpool.tile([128, W], f32, tag=f"m{qi}")
            nc.vector.tensor_tensor(m_t[:, :], u_t[:, :], v_t[:, :], op=OP.min)
            w_t = wpool.tile([128, W], bf16, tag=f"w{qi}")
            nc.vector.tensor_scalar(w_t[:, :], m_t[:, :], 0.0, None, op0=OP.max)
            w_tiles.append(w_t)
        w_y, w_x = w_tiles
        eng_clock[0] += 0.27 + 2 * 0.9  # sigmoid + 2 Copy-act v builds
        eng_clock[1] += 2 * (0.6 + 1.1 + 0.4)  # u/min/relu per axis
        if g == 0:
            for gg in range(1, GROUPS):
                gather(gg)

        # V[j, y] = sum_i probs[i, j] * WyT[i, y]
        v_ps = ps_v.tile([128, W], f32, tag="v_ps")
        for b in range(4):
            for (c0, cn) in V_CH:
                nc.tensor.matmul(
                    out=v_ps[32 * b : 32 * b + M, c0 : c0 + cn],
                    lhsT=probs[32 * b : 32 * b + M, :],
                    rhs=w_y[32 * b : 32 * b + M, c0 : c0 + cn],
                    start=True,
                    stop=True,
                    tile_position=(32 * b, 32 * b),
                )
        # split the V evacuation across both PSUM-capable engines
        v_sb = vpool.tile([128, W], bf16, tag="v_sb")
        nc.scalar.copy(v_sb[:, : W // 2], v_ps[:, : W // 2])
        nc.vector.tensor_copy(v_sb[:, W // 2 :], v_ps[:, W // 2 :])
        eng_clock[0] += 0.43
        eng_clock[1] += 0.53

        # out[y, x] = sum_j V[j, y] * WxT[j, x]; one PSUM tile per instance
        # (3 bufs) so next-tile matmuls never wait on this tile's evacuation
        for t in range(TILES):
            st = stage.tile([128, 4 * W], bf16, tag="st")
            for b in range(4):
                o_ps = ps_o.tile([128, W], f32, tag="o_ps")
                for (c0, cn) in V_CH:
                    nc.tensor.matmul(
                        out=o_ps[:, c0 : c0 + cn],
                        lhsT=v_sb[32 * b : 32 * b + M, t * 128 : (t + 1) * 128],
                        rhs=w_x[32 * b : 32 * b + M, c0 : c0 + cn],
                        start=True,
                        stop=True,
                        tile_position=(32 * b, 0),
                    )
                copy_psum(st[:, b * W : (b + 1) * W], o_ps[:, :], 0.85, 1.05)
            nc.sync.dma_start(
                out[4 * g : 4 * g + 4, t * 128 : (t + 1) * 128, :].rearrange(
                    "n y x -> y n x"
                ),
                st[:, :],
            )


def _build_program():
    import concourse.tile as tile
    from concourse import bacc, mybir
    from contextlib import ExitStack

    f32 = mybir.dt.float32
    bf16 = mybir.dt.bfloat16
    i32 = mybir.dt.int32

    nc = bacc.Bacc("TRN2", target_bir_lowering=False, debug=False)
    masks = nc.dram_tensor("masks", [N_LOC, C, M, M], f32, kind="ExternalInput").ap()
    offs = nc.dram_tensor("offs", [128, GROUPS], i32, kind="ExternalInput").ap()
    wvals = nc.dram_tensor("wvals", [128, 8 * GROUPS], f32, kind="ExternalInput").ap()
    iota = nc.dram_tensor("iota", [128, W], f32, kind="ExternalInput").ap()
    out = nc.dram_tensor("out", [N_LOC, H, W], bf16, kind="ExternalOutput").ap()

    with tile.TileContext(nc) as tc:
        with ExitStack() as ctx:
            tc._emit_ctx = ctx
            _emit(tc, nc, masks, offs, wvals, iota, out)
    nc.compile()
    return nc


_NC = None


def _get_program():
    global _NC
    if _NC is None:
        _NC = _build_program()
    return _NC


def _host_scalars(cls16, bbox16):
    """Per-core [128, k] tensors: gather row offsets + weight scalars."""
    p = np.arange(128)
    b = p // 32  # instance-in-group
    k = p % 32  # mask row / interp index per partition
    kcl = np.minimum(k, M - 1)

    cls = cls16.astype(np.int64)
    valid = (cls >= 0) & (cls < NUM_VALID)
    ccl = np.clip(cls, 0, C - 1)
    row_base = (np.arange(N_LOC) * C + ccl) * M  # [16]

    offs = np.empty((128, GROUPS), dtype=np.int32)
    wvals = np.empty((128, 8 * GROUPS), dtype=np.float32)
    pad = k >= M
    for g in range(GROUPS):
        n = 4 * g + b  # [128] instance ids
        offs[:, g] = row_base[n] + kcl
        for qi, (c0i, c1i) in enumerate(((1, 3), (0, 2))):  # y=(y0,y1), x=(x0,x1)
            s0 = bbox16[n, c0i]
            s1 = bbox16[n, c1i]
            ra = (s1 - s0) / M
            a = M / (s1 - s0)
            ck = (s0 - 0.5) + (k + 0.5) * ra
            ck = np.where(pad | ~valid[n], 1.0e9, ck)
            cb = 8 * g + 4 * qi
            # w = relu(min(u, v)); u = -a*s + (1 + a*c), v = a*s + (1 - a*c)
            wvals[:, cb + 0] = -a
            wvals[:, cb + 1] = 1.0 + a * ck
            wvals[:, cb + 2] = a
            wvals[:, cb + 3] = 1.0 - a * ck
    return offs, wvals


def make_in_maps(mask_output, class_indices, bbox_tensor):
    mask_output = np.asarray(mask_output, dtype=np.float32)
    class_indices = np.asarray(class_indices)
    bbox_tensor = np.asarray(bbox_tensor, dtype=np.float32)
    iota = np.broadcast_to(
        np.arange(W, dtype=np.float32)[None, :], (128, W)
    ).copy()
    in_maps = []
    for cidx in range(N_CORES):
        sl = slice(cidx * N_LOC, (cidx + 1) * N_LOC)
        offs, wvals = _host_scalars(class_indices[sl], bbox_tensor[sl])
        in_maps.append(
            {
                "masks": np.ascontiguousarray(mask_output[sl]),
                "offs": offs,
                "wvals": wvals,
                "iota": iota,
            }
        )
    return in_maps


def kernel(mask_output, class_indices, bbox_tensor, scene_h=H, scene_w=W, **kwargs):
    assert int(scene_h) == H and int(scene_w) == W
    from concourse.bass_utils import run_bass_kernel_spmd

    nc = _get_program()
    in_maps = make_in_maps(mask_output, class_indices, bbox_tensor)
    res = run_bass_kernel_spmd(nc, in_maps, list(range(N_CORES)))
    out = np.concatenate([np.asarray(r["out"]) for r in res.results], axis=0)
    return out.astype(np.float32)
